# revision 51
# baseline (speedup 1.0000x reference)
"""LogSparseAttention Trainium2 kernel (8-core SPMD), v3 pipelined.

Sharding: 8 cores = 2 batches x 4 head-groups (4 heads = 256 channels each).
Per core: causal convs (q, k) over full-D input for its 256 channels, v
projection, local window-16 attention + 8 exponential-jump terms for its 4
heads, partial output projection over its 256 channels. Host sums the 8
partial [D, T] outputs (4 per batch) and adds p_b.

v9 design (654us v2 baseline -> ~625us; HW has +-3-19% run-to-run
thermal/HAM variance, so compare medians):
  - work-unit scheduler: every non-conv PE op (v-projections, scores, po,
    pj, proj) is emitted INTO the conv matmul stream at ktile boundaries,
    one unit per ~6.8us of queued conv work, so its cross-engine deps (ACT
    exp, DVE mask/tmp) resolve long before the strict-FIFO PE queue reaches
    it.  Conv runs gap-free 0..~530us at full clock (217ns/512-col mm).
  - sync HW DMA queue carries ONLY the conv weight stream; everything else
    (inputs, consts, broadcasts, erows) lives on the scalar queue.  A 1MB
    replicated-read broadcast on sync stalls the weight stream ~10us.
  - DVE and GpSimd thrash each other ~3.5x when run concurrently (measured
    335ns -> 1290ns per op), so the jump-FMA runs single-engine: g0 fully
    on GpSimd under g1's conv (pre-normalized alpha rows, broadcast once
    per 512-chunk as one [64,E,512] DMA), g1 fully on DVE in the tail.
  - g1 tail: quarter exp-rows are broadcast UNNORMALIZED as soon as each
    jump quarter finishes (no wait on the global softmax sum); 1/Z is a
    [128,E] per-partition table built on-chip (mask-mult + select-matmul —
    a DRAM roundtrip costs ~20us of serial DMA latency) and applied via
    fast 4x-mode tensor_scalar pre-scales (chunk 0) or folded into the
    normalized chunk 1-3 broadcasts issued at jfin.  fma/proj pipeline per
    512-token chunk; conv(1,1) runs its k-pass first so jump quarters 0-1
    complete under the conv.
  - jump/output path is bf16 (f16 TT fell to 1x DVE mode); conv/q/k/scores
    stay f16 for precision (fp8/DoubleRow fails the 2e-2 gate: measured
    0.075-0.12 end-to-end even with one-side residual correction).
  - v_dt carries a 128-col wraparound extension so every shifted jump read
    is one full-width op; PSUM->SBUF drains for z and y are on ACT.

Layouts on chip (partition dim first):
  xT      8 x [128, 2063] f16     x transposed, left-padded 15 zeros
  q_sb    2 x [128, 2048] f16     conv out (+bias, x1/8)
  k_sb    2 x [128, 2063] f16     conv out (+bias), left-padded zeros
  v_dt    2 x [128, 2176] bf16    v (+bias), channel-partition, +128 wrap
  v_td    17 x [128, 4, 66] bf16  v (+bias) token-partition, +15-shifted,
                                  per-head ones column (Z accumulator)
  z       2 x [128, 2048] bf16    local attention output [d, t]
  zr      2 x [128, 2048] bf16    jump accumulator + z fold (proj input)
"""
import sys

sys.path.insert(0, "/opt/trn_rl_repo")

import collections
import numpy as np
import ml_dtypes
import concourse.bass as bass
import concourse.bacc as bacc
import concourse.tile as tile
from concourse import mybir

f32 = mybir.dt.float32
f16 = mybir.dt.float16
bf16 = mybir.dt.bfloat16
AL = mybir.AluOpType
AF = mybir.ActivationFunctionType

B, T, D = 2, 2048, 1024
H, W, E = 16, 16, 8
HD = D // H                  # 64
NCORES = 8
HPC = 4                      # heads per core
CH = HPC * HD                # 256 channels per core
NG = 2                       # head-pairs per core, 128 channels each
TP = W - 1                   # 15 (left pad)
TPAD = T + TP                # 2063
NT128 = T // 128             # 16
NT512 = T // 512             # 4
KT = D // 128                # 8 k-tiles over the input dim
SCALE = 1.0 / float(np.sqrt(HD))
MASKVAL = -200.0
_MICROBENCH = False

_CACHE = {}


def build_program():
    if "nc" in _CACHE:
        return _CACHE["nc"]
    import contextlib
    nc = bacc.Bacc()

    xT = nc.dram_tensor("xT", [D, TPAD], f16, kind="ExternalInput")
    # weights packed [g, ktile, in-ch-in-tile, (tap, out-ch)] for 4KB DMA lines
    qw = nc.dram_tensor("qw", [NG, KT, 128, W * 128], f16, kind="ExternalInput")
    kw = nc.dram_tensor("kw", [NG, KT, 128, W * 128], f16, kind="ExternalInput")
    vw = nc.dram_tensor("vw", [D, CH], f16, kind="ExternalInput")
    pw = nc.dram_tensor("pw", [CH, D], bf16, kind="ExternalInput")
    qb = nc.dram_tensor("qb", [CH, 1], f32, kind="ExternalInput")
    kb = nc.dram_tensor("kb", [CH, 1], f32, kind="ExternalInput")
    vb = nc.dram_tensor("vb", [CH, 1], f32, kind="ExternalInput")
    mask = nc.dram_tensor("mask", [272, 256], f32, kind="ExternalInput")
    ident = nc.dram_tensor("ident", [128, 128], bf16, kind="ExternalInput")
    onesp = nc.dram_tensor("onesp", [E, 128, 2 * E], f16, kind="ExternalInput")
    ones4 = nc.dram_tensor("ones4", [128, 2 * HPC], bf16, kind="ExternalInput")
    zpad = nc.dram_tensor("zpad", [128, TP], f16, kind="ExternalInput")
    vbrow = nc.dram_tensor("vbrow", [1, CH], bf16, kind="ExternalInput")
    vzero = nc.dram_tensor("vzero", [TP, CH], bf16, kind="ExternalInput")
    sel2 = nc.dram_tensor("sel2", [2 * E, 128], f32, kind="ExternalInput")
    m16e = nc.dram_tensor("m16e", [2 * E, E], f32, kind="ExternalInput")
    y = nc.dram_tensor("y", [D, T], f16, kind="ExternalOutput")
    erows_d = nc.dram_tensor("erows_d", [2 * E, T], bf16)
    alpha_d = nc.dram_tensor("alpha_d", [2 * E, T], bf16)

    with tile.TileContext(nc) as tc:
        with contextlib.ExitStack() as ctx:
            consts = ctx.enter_context(tc.tile_pool(name="consts", bufs=1))
            main = ctx.enter_context(tc.tile_pool(name="main", bufs=1))

            # ---- TEMP: DVE-mode micro-benchmark (remove after measuring) ----
            if _MICROBENCH:
                with tc.tile_pool(name="mb", bufs=1) as mb:
                    bA = mb.tile([128, 4096], bf16, name="mbA")
                    bB = mb.tile([128, 2176], bf16, name="mbB")
                    fA = mb.tile([128, 4096], f16, name="mfA")
                    fB = mb.tile([128, 2176], f16, name="mfB")
                    sA = mb.tile([128, 512], bf16, name="msA")
                    sB = mb.tile([128, 512], bf16, name="msB")
                    ot = [mb.tile([128, 512], bf16, name=f"mo{i}") for i in range(3)]
                    of = [mb.tile([128, 512], f16, name=f"mof{i}") for i in range(3)]
                    sep = mb.tile([128, 1], f32, name="msep")
                    for t in (bA, bB, fA, fB, sA, sB, sep):
                        nc.gpsimd.memset(t[:], 0.5)

                    def group(fn):
                        for i in range(3):
                            fn(i)
                        nc.vector.reciprocal(sep[:], sep[:])
                    nc.vector.reciprocal(sep[:], sep[:])
                    # 1: bf16 big-slice mult (fma replica)
                    group(lambda i: nc.vector.tensor_tensor(
                        ot[i][:], bA[:, 1024:1536], bB[:, 4:516], AL.mult))
                    # 2: bf16 big-slice add
                    group(lambda i: nc.vector.tensor_tensor(
                        ot[i][:], bA[:, 1024:1536], bB[:, 4:516], AL.add))
                    # 3: bf16 small-tile mult
                    group(lambda i: nc.vector.tensor_tensor(
                        ot[i][:], sA[:], sB[:], AL.mult))
                    # 4: f16 big-slice mult (jqm replica)
                    group(lambda i: nc.vector.tensor_tensor(
                        of[i][:], fA[:, 1024:1536], fB[:, 4:516], AL.mult))
                    # 5: bf16 big offset-0 mult
                    group(lambda i: nc.vector.tensor_tensor(
                        ot[i][:], bA[:, 0:512], bB[:, 0:512], AL.mult))
                    # 6: bf16 in-place mult
                    group(lambda i: nc.vector.tensor_tensor(
                        ot[i][:], ot[i][:], sB[:], AL.mult))
                    # 7: bf16 stt-mult
                    group(lambda i: nc.vector.scalar_tensor_tensor(
                        ot[i][:], sA[:], 1.0, sB[:], op0=AL.mult, op1=AL.mult))
                    # 8: bf16 mult, B offset 1024B
                    group(lambda i: nc.vector.tensor_tensor(
                        ot[i][:], bA[:, 1024:1536], bB[:, 512:1024], AL.mult))

            # ---- everything on the scalar queue: sync is reserved for the
            # conv weight stream so the PE's first conv matmul can start
            # within ~2us of kernel start ----
            xT_sb = [main.tile([128, TPAD], f16, tag=f"x{i}", name=f"xT_sb{i}") for i in range(KT)]
            vw_sb = [consts.tile([128, CH], f16, tag=f"vw{i}", name=f"vw_sb{i}") for i in range(KT)]
            vbt = consts.tile([128, CH], bf16)
            for i in range(KT):
                nc.scalar.dma_start(vw_sb[i][:], vw[128 * i:128 * (i + 1), :])
            nc.scalar.dma_start(vbt[:], vbrow[:].to_broadcast((128, CH)))
            for i in range(KT):
                nc.scalar.dma_start(xT_sb[i][:], xT[128 * i:128 * (i + 1), :])

            # ---- remaining constants ----
            m0 = consts.tile([128, 256], f32)
            m1 = consts.tile([128, 256], f32)
            m2 = consts.tile([TP, 256], f32)
            nc.scalar.dma_start(m0[:], mask[0:128, :])
            nc.scalar.dma_start(m1[:], mask[128:256, :])
            nc.scalar.dma_start(m2[:], mask[256:271, :])
            id_sb = consts.tile([128, 128], bf16)
            nc.scalar.dma_start(id_sb[:], ident[:])
            onesp_sb = consts.tile([128, E, 2 * E], f16)
            nc.scalar.dma_start(onesp_sb[:], onesp.rearrange("e p m -> p e m"))
            qb_sb = consts.tile([128, NG], f32)
            kb_sb = consts.tile([128, NG], f32)
            vb_sb = consts.tile([128, NG], f32)
            nc.scalar.dma_start(qb_sb[:], qb.rearrange("(g p) o -> p (g o)", g=NG))
            nc.scalar.dma_start(kb_sb[:], kb.rearrange("(g p) o -> p (g o)", g=NG))
            nc.scalar.dma_start(vb_sb[:], vb.rearrange("(g p) o -> p (g o)", g=NG))
            pw_sb = [consts.tile([128, D], bf16, tag=f"pw{g}", name=f"pw_sb{g}") for g in range(NG)]
            for g in range(NG):
                nc.scalar.dma_start(pw_sb[g][:], pw[128 * g:128 * (g + 1), :])
            sel2_sb = consts.tile([2 * E, 128], f32)
            m16e_sb = consts.tile([2 * E, E], f32)
            nc.scalar.dma_start(sel2_sb[:], sel2[:])
            nc.scalar.dma_start(m16e_sb[:], m16e[:])
            q_sb = [main.tile([128, T], f16, tag=f"q{g}", name=f"q_sb{g}") for g in range(NG)]
            k_sb = [main.tile([128, TPAD], f16, tag=f"k{g}", name=f"k_sb{g}") for g in range(NG)]
            v_dt = [main.tile([128, T + 128], bf16, tag=f"vdt{g}", name=f"v_dt{g}") for g in range(NG)]
            v_td = [main.tile([128, HPC, HD + 2], bf16, tag=f"vtd{j}", name=f"v_td{j}")
                    for j in range(NT128 + 1)]
            z = [main.tile([128, T], bf16, tag=f"z{g}", name=f"z{g}") for g in range(NG)]
            zr = [main.tile([128, T], bf16, tag=f"zr{g}", name=f"zr{g}") for g in range(NG)]
            arows = main.tile([2 * E, T], bf16, tag="ar", name="arows")

            for g in range(NG):
                nc.scalar.dma_start(k_sb[g][:, 0:TP], zpad[:])

            # ===== phases 1-4: conv stream with interleaved work units =====
            with tc.tile_pool(name="attn", bufs=1) as apool, \
                 tc.tile_pool(name="wstream", bufs=6) as wpool, \
                 tc.tile_pool(name="psC", bufs=1, space="PSUM") as psC, \
                 tc.tile_pool(name="psA", bufs=1, space="PSUM") as psA:

                erows_sh = apool.tile([2 * E, T], bf16, tag="erows",
                                      name="erows")
                erows = [erows_sh, erows_sh]
                exp_store = {}
                tmp_store = {}

                def vdt_unit(g, t4):
                    # v in [d, t] layout (M=d), with bias
                    def run():
                        pv2 = psA.tile([128, 512], f32, tag="score", bufs=3,
                                       name=f"pv2_{g}_{t4}")
                        for i in range(KT):
                            nc.tensor.matmul(
                                pv2[:],
                                vw_sb[i][:, 128 * g:128 * (g + 1)],
                                xT_sb[i][:, TP + 512 * t4: TP + 512 * (t4 + 1)],
                                start=(i == 0), stop=(i == KT - 1),
                            )
                        nc.vector.tensor_scalar(
                            v_dt[g][:, 512 * t4:512 * (t4 + 1)], pv2[:],
                            vb_sb[:, g:g + 1], None, op0=AL.add,
                        )
                        if t4 == 0:
                            # wraparound extension [T:T+128] = v[:, 0:128]
                            nc.scalar.dma_start(
                                v_dt[g][:, T:T + 128], v_dt[g][:, 0:128])
                    return run

                def vtd_unit(j):
                    # v in [t, d] layout (M=t), shifted +15, with bias
                    def run():
                        mrow = 128 if j < NT128 else TP
                        pv = psA.tile([128, 256], f32, tag="score", bufs=3,
                                      name=f"pv{j}")
                        for i in range(KT):
                            nc.tensor.matmul(
                                pv[0:mrow, :],
                                xT_sb[i][:, 128 * j:128 * j + mrow],
                                vw_sb[i][:],
                                start=(i == 0), stop=(i == KT - 1),
                            )
                        nc.vector.tensor_tensor(
                            v_td[j][0:mrow, :, 0:HD],
                            pv[0:mrow, :].rearrange("p (h d) -> p h d", h=HPC),
                            vbt[0:mrow, :].rearrange("p (h d) -> p h d", h=HPC),
                            AL.add,
                        )
                        if j == 0:
                            nc.scalar.dma_start(
                                v_td[0][0:TP, :, 0:HD],
                                vzero.rearrange("p (h d) -> p h d", h=HPC))
                        nc.scalar.dma_start(
                            v_td[j][:, :, HD:HD + 2],
                            ones4.rearrange("p (h o) -> p h o", o=2))
                    return run

                def conv_pair(g, p, units, k_first=False, split_q=False):
                    # one weight stream covers two 512-token chunks; q-pass
                    # then k-pass, 2 PSUM banks total.  After each ktile
                    # (~6.8us of queued PE work) one work unit is emitted.
                    # k_first finishes k at halftime; split_q additionally
                    # runs the q-pass one chunk at a time (re-streaming the
                    # q weights once, +4MB DMA) so q[1024:1536] drains at
                    # ~75% and more attention work leaves the tail.
                    if split_q:
                        passes = [(kw, False, (0, 1)), (qw, True, (0,)),
                                  (qw, True, (1,))]
                    elif k_first:
                        passes = [(kw, False, (0, 1)), (qw, True, (0, 1))]
                    else:
                        passes = [(qw, True, (0, 1)), (kw, False, (0, 1))]
                    for wdram, is_q, chunks in passes:
                        pcs = {c: psC.tile([128, 512], f32, tag=f"cv{c}",
                                           bufs=1, name=f"pc{g}_{p}_{c}")
                               for c in chunks}
                        for i in range(KT):
                            wt = wpool.tile([128, W * 128], f16,
                                            tag=f"w{int(is_q)}", bufs=3)
                            nc.sync.dma_start(wt[:], wdram[g, i])
                            for dt in range(W):
                                first = (i == 0 and dt == 0)
                                last = (i == KT - 1 and dt == W - 1)
                                for c in chunks:
                                    c0 = 512 * (2 * p + c) + dt
                                    nc.tensor.matmul(
                                        pcs[c][:], wt[:, 128 * dt:128 * (dt + 1)],
                                        xT_sb[i][:, c0:c0 + 512],
                                        start=first, stop=last)
                            if units:
                                units.popleft()()
                        for c in chunks:
                            t0 = 512 * (2 * p + c)
                            if is_q:
                                nc.vector.tensor_scalar(
                                    q_sb[g][:, t0:t0 + 512], pcs[c][:],
                                    qb_sb[:, g:g + 1], SCALE,
                                    op0=AL.add, op1=AL.mult)
                            else:
                                nc.vector.tensor_scalar(
                                    k_sb[g][:, TP + t0:TP + t0 + 512], pcs[c][:],
                                    kb_sb[:, g:g + 1], None, op0=AL.add)
                    while units:
                        units.popleft()()

                def jqm_unit(g, q4):
                    # DVE mults for one jump-score quarter; pj matmuls come
                    # one slot later so the PE never waits on these.
                    def run():
                        tmps = []
                        for e in range(E):
                            sh = 1 << e
                            t0 = 512 * q4
                            tmp = apool.tile([128, 512], f16, tag="jtmp",
                                             bufs=8, name="jtmp")
                            ov = max(0, t0 + 512 + sh - T)  # wrap amount
                            nc.vector.tensor_tensor(
                                tmp[:, 0:512 - ov],
                                q_sb[g][:, t0:t0 + 512 - ov],
                                k_sb[g][:, TP + t0 + sh:TP + t0 + sh + 512 - ov],
                                AL.mult)
                            if ov:
                                nc.vector.tensor_tensor(
                                    tmp[:, 512 - ov:512],
                                    q_sb[g][:, t0 + 512 - ov:t0 + 512],
                                    k_sb[g][:, TP:TP + ov], AL.mult)
                            tmps.append(tmp)
                        tmp_store[(g, q4)] = tmps
                    return run

                ebc_store = {}
                zsq_store = {}
                rz_store = {}

                def jqp_unit(g, q4):
                    # jump-score quarter: matmul-reduce, exp, partial Z sum,
                    # and the (unnormalized) exp-row broadcast for this
                    # quarter — so no broadcast traffic waits on the global
                    # softmax sum at the tail.
                    def run():
                        tmps = tmp_store.pop((g, q4))
                        pj = psA.tile([2 * E, 512], f32, tag="pj", bufs=1,
                                      name="pj")
                        for e in range(E):
                            nc.tensor.matmul(
                                pj[:], onesp_sb[:, e, :], tmps[e][:],
                                start=(e == 0), stop=(e == E - 1))
                        t0 = 512 * q4
                        sl = slice(t0, t0 + 512)
                        nc.scalar.activation(erows[g][:, sl], pj[:], AF.Exp)
                        if q4 == 0:
                            zsq_store[g] = apool.tile(
                                [2 * E, NT512], f32, tag="zsq", bufs=2,
                                name=f"zsq{g}")
                        nc.vector.tensor_reduce(
                            zsq_store[g][:, q4:q4 + 1], erows[g][:, sl],
                            mybir.AxisListType.X, AL.add)
                        if g == 0 or q4 >= 1:
                            # g0 and g1's chunks 1-3 use normalized
                            # broadcasts issued at their jfin instead
                            return
                        # g1 chunk 0: broadcast the UNNORMALIZED exp rows now
                        # (under g1's conv) so the tail's first fma chunk has
                        # its broadcast long in flight; 1/Z is applied via
                        # per-e prescales.  Scalar queue: the 1MB replicated
                        # broadcast must never block the weight stream (sync).
                        nc.scalar.dma_start(erows_d[:, sl], erows[g][:, sl])
                        src = erows_d.rearrange("(e two) t -> two e t", two=2)
                        ebc = apool.tile([128, E * 512], bf16, tag="ebc",
                                         bufs=4, name=f"ebc{g}_{q4}")
                        for half in range(2):
                            nc.scalar.dma_start(
                                ebc[64 * half:64 * half + 64]
                                .rearrange("p (e t) -> p e t", e=E),
                                src[half, :, t0:t0 + 512]
                                .rearrange("e t -> () e t")
                                .to_broadcast((64, E, 512)))
                        ebc_store[(g, q4)] = ebc
                    return run

                def jfin_unit(g):
                    def run():
                        zsum = apool.tile([2 * E, 1], f32, tag="zsum", name="zsum")
                        nc.vector.tensor_reduce(
                            zsum[:], zsq_store[g][:], mybir.AxisListType.X,
                            AL.add)
                        nc.vector.reciprocal(zsum[:], zsum[:])
                        # normalized alpha rows -> chunked broadcasts.
                        # g0 (GpSimd fma, under conv): all 4 chunks.
                        # g1 (DVE fma, tail): chunks 1-3 — chunk 0 was
                        # broadcast unnormalized under the conv so the
                        # tail's first fma never waits on this.
                        nc.vector.tensor_scalar(
                            arows[:], erows[g][:], zsum[:, 0:1], None,
                            op0=AL.mult)
                        nc.scalar.dma_start(alpha_d[:], arows[:])
                        src = alpha_d.rearrange("(e two) t -> two e t", two=2)
                        for c in range(0 if g == 0 else 1, NT512):
                            t0 = 512 * c
                            bca = apool.tile(
                                [128, E * 512], bf16, tag="ebc", bufs=4,
                                name=f"bca{g}_{c}")
                            eng = nc.scalar if (g == 0 or c % 2 == 0) \
                                else nc.sync
                            for half in range(2):
                                eng.dma_start(
                                    bca[64 * half:64 * half + 64]
                                    .rearrange("p (e t) -> p e t", e=E),
                                    src[half, :, t0:t0 + 512]
                                    .rearrange("e t -> () e t")
                                    .to_broadcast((64, E, 512)))
                            ebc_store[(g, c)] = bca
                        if g == 0:
                            return
                        # g1 chunk 0 still needs 1/Z as a [128, E] per-
                        # partition scalar table, built fully on-chip (a DRAM
                        # roundtrip here costs ~20us of serial DMA latency on
                        # the tail critical path)
                        zb = apool.tile([2 * E, E], f32, tag="zb", bufs=2,
                                        name=f"zb{g}")
                        nc.vector.tensor_scalar(
                            zb[:], m16e_sb[:], zsum[:, 0:1], None, op0=AL.mult)
                        rp = psA.tile([128, E], f32, tag="popt", bufs=2,
                                      name=f"rzp{g}")
                        nc.tensor.matmul(rp[:], sel2_sb[:], zb[:],
                                         start=True, stop=True)
                        rzall = apool.tile([128, E], f32, tag="rzall", bufs=2,
                                           name=f"rzall{g}")
                        nc.scalar.activation(rzall[:], rp[:], AF.Copy)
                        rz_store[g] = rzall
                    return run

                def fma_unit(g, c):
                    # sum_e alpha_e (*) v_shifted for one 512-token chunk,
                    # accumulated straight into zr (z folded at the end).
                    # Single-engine by design: DVE and GpSimd thrash each
                    # other ~3.5x when run concurrently, so g0 runs entirely
                    # on GpSimd (hidden under g1's conv, pre-normalized
                    # alpha) and g1 entirely on DVE (tail, 1/Z applied via
                    # fast tensor_scalar pre-scales of v).
                    def run():
                        t0 = 512 * c
                        sl = slice(t0, t0 + 512)
                        ebc = ebc_store[(g, c)]
                        acc = zr[g]
                        first = True
                        for e in range(E):
                            sh = 1 << e
                            src = v_dt[g][:, t0 + sh:t0 + sh + 512]
                            eb = ebc[:, 512 * e:512 * (e + 1)]
                            if g == 0:
                                eng = nc.gpsimd
                            else:
                                eng = nc.vector
                                if c == 0:
                                    # only chunk 0's broadcast is
                                    # unnormalized; apply 1/Z here
                                    vs = apool.tile([128, 512], bf16,
                                                    tag="vs", bufs=3,
                                                    name="vs")
                                    nc.vector.tensor_scalar(
                                        vs[:], src, rz_store[g][:, e:e + 1],
                                        None, op0=AL.mult)
                                    src = vs[:]
                            if first:
                                eng.tensor_tensor(acc[:, sl], eb, src, AL.mult)
                                first = False
                            else:
                                ft = apool.tile([128, 512], bf16,
                                                tag=f"ft{g}", bufs=2, name="ft")
                                eng.tensor_tensor(ft[:], eb, src, AL.mult)
                                eng.tensor_tensor(
                                    acc[:, sl], acc[:, sl], ft[:], AL.add)
                        eng.tensor_tensor(
                            acc[:, sl], acc[:, sl], z[g][:, sl], AL.add)
                    return run

                def sc_unit(g, c):
                    def run():
                        exps = []
                        for hh in range(2):
                            r0, r1 = 64 * hh, 64 * hh + 64
                            e0 = apool.tile([128, 256], bf16, tag="e0", bufs=3, name="e0")
                            e1 = apool.tile([128, 256], bf16, tag="e1", bufs=3, name="e1")
                            e2 = apool.tile([TP, 256], bf16, tag="e2", bufs=3, name="e2")
                            for (et, msk, s0, srows) in (
                                (e0, m0, 256 * c, 128),
                                (e1, m1, 256 * c + 128, 128),
                                (e2, m2, 256 * c + 256, TP),
                            ):
                                ps = psA.tile([128, 256], f32, tag="score", bufs=3, name="ps")
                                nc.tensor.matmul(
                                    ps[0:srows, :],
                                    k_sb[g][r0:r1, s0:s0 + srows],
                                    q_sb[g][r0:r1, 256 * c:256 * (c + 1)],
                                    start=True, stop=True,
                                )
                                nc.vector.tensor_tensor(
                                    ps[0:srows, :], ps[0:srows, :],
                                    msk[0:srows, :], AL.add)
                                nc.scalar.activation(
                                    et[0:srows, :], ps[0:srows, :], AF.Exp)
                            exps.append((e0, e1, e2))
                        exp_store[(g, c)] = exps
                    return run

                def po_unit(g, c):
                    def run():
                        exps = exp_store.pop((g, c))
                        for sub in range(2):
                            jj = 2 * c + sub
                            stage = apool.tile([128, 128], bf16, tag="stage", bufs=2, name="stage")
                            for hh in range(2):
                                e0, e1, e2 = exps[hh]
                                if sub == 0:
                                    lo, hi = e0[:, 0:128], e1[0:TP, 0:128]
                                else:
                                    lo, hi = e1[:, 128:256], e2[0:TP, 128:256]
                                hl = 2 * g + hh
                                po = psA.tile([128, HD + 2], f32, tag="popt", bufs=2, name="po")
                                nc.tensor.matmul(
                                    po[:], lo, v_td[jj][:, hl, :],
                                    start=True, stop=False,
                                )
                                nc.tensor.matmul(
                                    po[:], hi, v_td[jj + 1][0:TP, hl, :],
                                    start=False, stop=True,
                                )
                                rz = apool.tile([128, 1], f32, tag="rz", bufs=2, name="rz")
                                nc.vector.reciprocal(rz[:], po[:, HD:HD + 1])
                                nc.vector.tensor_scalar(
                                    stage[:, 64 * hh:64 * hh + 64],
                                    po[:, 0:HD], rz[:], None, op0=AL.mult,
                                )
                            pt = psA.tile([128, 128], bf16, tag="popt", bufs=2, name="pt")
                            nc.tensor.transpose(pt[:], stage[:], id_sb[:])
                            tcol = 256 * c + 128 * sub
                            nc.scalar.activation(
                                z[g][:, tcol:tcol + 128], pt[:], AF.Copy)
                    return run

                def proj_unit(t4):
                    def run():
                        for o8 in range(D // 128):
                            # conv is done by proj time: reuse its two PSUM
                            # banks (same shape/tag) as a double buffer
                            py = psC.tile([128, 512], f32, tag=f"cv{o8 % 2}",
                                          bufs=1, name="py")
                            for g in range(NG):
                                nc.tensor.matmul(
                                    py[:],
                                    pw_sb[g][:, 128 * o8:128 * (o8 + 1)],
                                    zr[g][:, 512 * t4:512 * (t4 + 1)],
                                    start=(g == 0), stop=(g == NG - 1),
                                )
                            ysb = apool.tile([128, 512], f16, tag="ysb", bufs=4, name="ysb")
                            nc.scalar.activation(ysb[:], py[:], AF.Copy)
                            eng = nc.sync if o8 % 2 == 0 else nc.scalar
                            eng.dma_start(
                                y[128 * o8:128 * (o8 + 1),
                                  512 * t4:512 * (t4 + 1)], ysb[:])
                    return run

                dq = collections.deque
                noop = lambda: None

                # v-projection units need the full xT (in flight for the
                # first ~2 slots); everything else is keyed to conv outputs
                # one pass or more old.
                conv_pair(0, 0, dq(
                    [noop, noop, noop] + [vtd_unit(j) for j in range(13)]))
                conv_pair(0, 1, dq([
                    vtd_unit(13), vtd_unit(14), vtd_unit(15), vtd_unit(16),
                    sc_unit(0, 0), po_unit(0, 0), sc_unit(0, 1), po_unit(0, 1),
                    sc_unit(0, 2), po_unit(0, 2), sc_unit(0, 3), po_unit(0, 3),
                    jqm_unit(0, 0), jqp_unit(0, 0),
                    vdt_unit(0, 0), vdt_unit(0, 1),
                ]))
                conv_pair(1, 0, dq([
                    vdt_unit(0, 2), vdt_unit(0, 3), vdt_unit(1, 0),
                    vdt_unit(1, 1), vdt_unit(1, 2), vdt_unit(1, 3),
                    sc_unit(0, 4), po_unit(0, 4), sc_unit(0, 5), po_unit(0, 5),
                    sc_unit(0, 6), po_unit(0, 6), sc_unit(0, 7), po_unit(0, 7),
                    jqm_unit(0, 1), jqp_unit(0, 1),
                ]))
                # last pair: k-pass first, then per-chunk q half-passes —
                # k(g1) completes at 1/3, q[1024:1536] at 2/3, so all jump
                # quarters but the last and local chunks 0-4 run under the
                # conv (24 unit slots)
                conv_pair(1, 1, dq([
                    jqm_unit(0, 2), jqp_unit(0, 2), jqm_unit(0, 3),
                    jqp_unit(0, 3), jfin_unit(0),
                    fma_unit(0, 0), fma_unit(0, 1), fma_unit(0, 2),
                    fma_unit(0, 3),
                    sc_unit(1, 0), po_unit(1, 0), sc_unit(1, 1), po_unit(1, 1),
                    sc_unit(1, 2), po_unit(1, 2), jqm_unit(1, 0),
                    jqp_unit(1, 0), jqm_unit(1, 1), jqp_unit(1, 1),
                    sc_unit(1, 3), po_unit(1, 3), jqm_unit(1, 2),
                    jqp_unit(1, 2), sc_unit(1, 4),
                ]), split_q=True)
                # tail: last jump quarter + jfin first (they gate every fma
                # chunk), then fma/proj pipelined per chunk with the
                # remaining local-attention chunks filling PE time
                for u in [
                    po_unit(1, 4), sc_unit(1, 5), po_unit(1, 5),
                    jqm_unit(1, 3), jqp_unit(1, 3), jfin_unit(1),
                    fma_unit(1, 0), sc_unit(1, 6), po_unit(1, 6),
                    fma_unit(1, 1), proj_unit(0), sc_unit(1, 7), po_unit(1, 7),
                    fma_unit(1, 2), proj_unit(1),
                    fma_unit(1, 3), proj_unit(2), proj_unit(3),
                ]:
                    u()

    nc.compile()
    _CACHE["nc"] = nc
    return nc


def make_consts():
    mask = np.full((272, 256), MASKVAL, np.float32)
    rel = np.arange(271)[:, None]
    trel = np.arange(256)[None, :]
    band = (rel >= trel) & (rel <= trel + TP)
    mask[:271][band] = 0.0
    ident = np.eye(128, dtype=ml_dtypes.bfloat16)
    onesp = np.zeros((E, 128, 2 * E), np.float16)
    for e in range(E):
        onesp[e, 0:64, 2 * e] = 1.0
        onesp[e, 64:128, 2 * e + 1] = 1.0
    ones4 = np.zeros((128, 2 * HPC), ml_dtypes.bfloat16)
    ones4[:, 0::2] = 1.0
    zpad = np.zeros((128, TP), np.float16)
    sel2 = np.zeros((2 * E, 128), np.float32)
    for r in range(2 * E):
        sel2[r, 64 * (r % 2):64 * (r % 2) + 64] = 1.0
    m16e = np.zeros((2 * E, E), np.float32)
    for r in range(2 * E):
        m16e[r, r // 2] = 1.0
    return mask, ident, onesp, ones4, zpad, sel2, m16e


def _pack_conv_w(wslice):
    # wslice [CH, D, W] (torch layout for this core's channels) ->
    # [NG, KT, 128, W*128]: [g, i, in-ch p, (tap dt, out-ch o)]
    a = wslice.reshape(NG, 128, KT, 128, W)         # [g, o, i, p, dt]
    a = a.transpose(0, 2, 3, 4, 1)                  # [g, i, p, dt, o]
    return np.ascontiguousarray(
        a.reshape(NG, KT, 128, W * 128).astype(np.float16))


def make_in_maps(x, q_w, q_b, k_w, k_b, v_w, v_b, p_w):
    mask, ident, onesp, ones4, zpad, sel2, m16e = make_consts()
    in_maps = []
    for core in range(NCORES):
        b, g = core // HPC, core % HPC
        ch = slice(CH * g, CH * (g + 1))
        xTf = np.zeros((D, TPAD), np.float16)
        xTf[:, TP:] = x[b].T.astype(np.float16)
        in_maps.append({
            "xT": xTf,
            "qw": _pack_conv_w(q_w[ch]),
            "kw": _pack_conv_w(k_w[ch]),
            "vw": np.ascontiguousarray(v_w[ch].T.astype(np.float16)),
            "pw": np.ascontiguousarray(
                p_w[:, ch].T.astype(ml_dtypes.bfloat16)),
            "qb": np.ascontiguousarray(q_b[ch][:, None].astype(np.float32)),
            "kb": np.ascontiguousarray(k_b[ch][:, None].astype(np.float32)),
            "vb": np.ascontiguousarray(v_b[ch][:, None].astype(np.float32)),
            "mask": mask, "ident": ident, "onesp": onesp,
            "ones4": ones4, "zpad": zpad, "sel2": sel2, "m16e": m16e,
            "vbrow": np.ascontiguousarray(
                v_b[ch][None, :].astype(ml_dtypes.bfloat16)),
            "vzero": np.zeros((TP, CH), ml_dtypes.bfloat16),
        })
    return in_maps


def assemble_output(results, p_b):
    out = np.zeros((B, T, D), np.float32)
    for core in range(NCORES):
        out[core // HPC] += results[core]["y"].T.astype(np.float32)
    out += p_b[None, None, :]
    return out


def _run(inputs, trace=False):
    from concourse.bass_utils import run_bass_kernel_spmd
    nc = build_program()
    args = {k: np.asarray(v, np.float32) for k, v in inputs.items()}
    p_b = args.pop("p_b")
    in_maps = make_in_maps(**args)
    res = run_bass_kernel_spmd(nc, in_maps, list(range(NCORES)), trace=trace)
    out = assemble_output(res.results, p_b)
    return out, res


def kernel(**inputs):
    out, _ = _run(inputs)
    return out
